# revision 11
# baseline (speedup 1.0000x reference)
"""Trainium2 Bass kernel for nn_CrossLayerV2 (MoE low-rank bilinear cross layer).

Computes, for x0,x [B,D], U [E,D,R], V [E,R,D], C [E,R,R], b [E,D], Wg [E,D], bg [E]:
    g    = softmax(x @ Wg.T + bg, axis=1)                 # [B, E]
    xvc  = einsum('bd,eds->bes', x, W1)  with W1[e] = V[e].T @ C[e]   (host-fused)
    out  = x0 * einsum('be,bes,eds->bd', g, xvc, U) + g @ b + x * g.sum(1, keepdims=True)
Note g.sum(1) == 1 (softmax), so the last term is exactly x.

Strategy: data-parallel over batch across 8 NeuronCores (params replicated).
Per core: 2048 rows = 4 blocks x (4 subtiles of 128 rows). All matmuls in bf16
with fp32 PSUM accumulation; the final residual add is fp32. x arrives twice:
fp32 (for the exact +x term) and bf16 (DMA-transposed into [D, B] layout for
the tensor-engine contractions).
"""

import sys

for _p in ("/opt/trn_rl_repo", "/opt/pypackages"):
    if _p not in sys.path:
        sys.path.append(_p)

from contextlib import ExitStack

import ml_dtypes
import numpy as np

import concourse.bass as bass
import concourse.tile as tile
from concourse import mybir
from concourse.bass_utils import run_bass_kernel_spmd
from concourse.masks import make_identity

BF16 = mybir.dt.bfloat16
F32 = mybir.dt.float32
NPBF16 = ml_dtypes.bfloat16

B, D, R, E = 16384, 512, 64, 8
NCORES = 8
BL = B // NCORES          # rows per core (2048)
P = 128                   # partitions
NSUB = 4                  # subtiles per block
BLOCK = NSUB * P          # rows per block (512)
NBLK = BL // BLOCK        # blocks per core (4)
KC = D // P               # contraction chunks (4)
NPAIR = E // 2            # expert pairs (4)
ES = E * R                # expert-packed width (512)


def _expand_gate_ap(g_ap: bass.AP, reps: int) -> bass.AP:
    """[P, E] gate AP -> broadcast [P, E, reps] AP (stride-0 inner dim)."""
    return bass.AP(tensor=g_ap.tensor, offset=g_ap.offset,
                   ap=[g_ap.ap[0], g_ap.ap[1], [0, reps]])


def _kernel_body(tc, out_d, x_d, x0_d, w1_d, wg_d, ut_d, b_d, bg_d):
    nc = tc.nc
    x_v = x_d.rearrange("(nb s p) d -> nb p s d", s=NSUB, p=P)
    x0_v = x0_d.rearrange("(nb s p) d -> nb p s d", s=NSUB, p=P)
    out_v = out_d.rearrange("(nb s p) d -> nb p s d", s=NSUB, p=P)

    with ExitStack() as ctx:
        const = ctx.enter_context(tc.tile_pool(name="const", bufs=1))
        io = ctx.enter_context(tc.tile_pool(name="io", bufs=2))
        outp = ctx.enter_context(tc.tile_pool(name="outp", bufs=2))
        work = ctx.enter_context(tc.tile_pool(name="work", bufs=2))
        small = ctx.enter_context(tc.tile_pool(name="small", bufs=4))
        ps_gate = ctx.enter_context(tc.tile_pool(name="ps_gate", bufs=2, space="PSUM"))
        ps_xvc = ctx.enter_context(tc.tile_pool(name="ps_xvc", bufs=2, space="PSUM"))
        ps_tp = ctx.enter_context(tc.tile_pool(name="ps_tp", bufs=2, space="PSUM"))
        ps_out = ctx.enter_context(tc.tile_pool(name="ps_out", bufs=2, space="PSUM"))

        # --- resident constants / params ---
        ident = const.tile([P, P], BF16)
        make_identity(nc, ident)
        identf = const.tile([P, P], F32)
        make_identity(nc, identf)
        ones_row = const.tile([1, BLOCK], BF16)
        nc.vector.memset(ones_row, 1.0)
        w1_sb = const.tile([P, KC, ES], BF16)
        nc.sync.dma_start(w1_sb, w1_d.rearrange("k p n -> p k n"))
        wg_sb = const.tile([P, KC, E], BF16)
        nc.sync.dma_start(wg_sb, wg_d.rearrange("k p n -> p k n"))
        ut_sb = const.tile([P, NPAIR, D], BF16)
        nc.sync.dma_start(ut_sb, ut_d.rearrange("k p n -> p k n"))
        b_sb = const.tile([E, D], BF16)
        nc.sync.dma_start(b_sb, b_d)
        bg_sb = const.tile([1, E], BF16)
        nc.sync.dma_start(bg_sb, bg_d)

        for blk in range(NBLK):
            x_t = io.tile([P, NSUB, D], F32, tag="x")
            nc.sync.dma_start(x_t, x_v[blk])
            x0_t = io.tile([P, NSUB, D], BF16, tag="x0")
            nc.sync.dma_start(x0_t, x0_v[blk])

            out_t = outp.tile([P, NSUB, D], F32, tag="o")

            def head(s):
                """x transpose + xvc matmuls + gate logits for subtile s."""
                # transpose x subtile on the PE (fp32 in, bf16 out via copy)
                p_xt = ps_tp.tile([P, BLOCK], F32, tag="tp")
                for k in range(KC):
                    nc.tensor.transpose(p_xt[:, k * P:(k + 1) * P],
                                        x_t[:, s, k * P:(k + 1) * P], identf)
                xt = work.tile([P, KC, P], BF16, tag="xt")
                nc.vector.tensor_copy(xt, p_xt)

                p_xvc = ps_xvc.tile([P, ES], F32, tag="xvc")
                for k in range(KC):
                    nc.tensor.matmul(p_xvc, xt[:, k, :], w1_sb[:, k, :],
                                     start=(k == 0), stop=(k == KC - 1))
                # gate logits, transposed: lgt[e, j] = sum_d Wg[e,d] x[j,d] + bg
                p_lgt = ps_gate.tile([E, P], F32, tag="gate")
                for k in range(KC):
                    nc.tensor.matmul(p_lgt, wg_sb[:, k, :], xt[:, k, :],
                                     start=(k == 0), stop=False)
                nc.tensor.matmul(p_lgt, bg_sb, ones_row[:, :P],
                                 start=False, stop=True)
                lgt_sb = small.tile([E, P], BF16, tag="lgt")
                nc.vector.tensor_copy(lgt_sb, p_lgt)
                p_log = ps_gate.tile([P, E], BF16, tag="gate")
                nc.tensor.transpose(p_log, lgt_sb, ident[:E, :E])
                return p_xvc, p_log

            state = {0: head(0)}

            for s in range(NSUB):
                ssl = slice(s * P, (s + 1) * P)
                p_xvc, p_log = state.pop(s)

                # softmax over E (free dim), produced directly in bf16
                nm = small.tile([P, 1], F32, tag="nm")
                nc.vector.reduce_max(nm, p_log, axis=mybir.AxisListType.X,
                                     negate=True)
                p_t = small.tile([P, E], F32, tag="pt")
                s_t = small.tile([P, 1], F32, tag="st")
                nc.scalar.activation(p_t, p_log, mybir.ActivationFunctionType.Exp,
                                     bias=nm[:, :], accum_out=s_t[:, :])
                r_t = small.tile([P, 1], F32, tag="rt")
                nc.vector.reciprocal(r_t, s_t)
                g_bf = small.tile([P, E], BF16, tag="g")
                nc.vector.tensor_scalar_mul(g_bf, p_t, r_t[:, :])

                # gxvc = xvc * g (gate broadcast along the rank dim), cast bf16
                gxvc = work.tile([P, ES], BF16, tag="gxvc")
                nc.vector.tensor_tensor(gxvc, p_xvc, _expand_gate_ap(g_bf[:, :], R),
                                        op=mybir.AluOpType.mult)

                # transpose expert-pair blocks + the gate row for g @ b
                p_gx = ps_tp.tile([P, ES], BF16, tag="tp")
                for pr in range(NPAIR):
                    nc.tensor.transpose(p_gx[:, pr * P:(pr + 1) * P],
                                        gxvc[:, pr * P:(pr + 1) * P], ident)
                p_gt = ps_tp.tile([E, P], BF16, tag="tp")
                nc.tensor.transpose(p_gt, g_bf, ident)
                gxt = work.tile([P, ES], BF16, tag="gxt")
                nc.scalar.copy(gxt, p_gx)
                gt_sb = small.tile([E, P], BF16, tag="gt")
                nc.scalar.copy(gt_sb, p_gt)

                # keep PE fed: issue the next subtile's contraction now
                if s + 1 < NSUB:
                    state[s + 1] = head(s + 1)

                # gproj = sum over expert pairs + g @ b
                p_o = ps_out.tile([P, D], F32, tag="out")
                for pr in range(NPAIR):
                    nc.tensor.matmul(p_o, gxt[:, pr * P:(pr + 1) * P],
                                     ut_sb[:, pr, :], start=(pr == 0), stop=False)
                nc.tensor.matmul(p_o, gt_sb, b_sb, start=False, stop=True)

                # out = x0 * gproj + x  (fp32 residual)
                tmp = work.tile([P, D], F32, tag="tmp")
                nc.vector.tensor_mul(tmp, p_o, x0_t[:, s, :])
                nc.gpsimd.tensor_add(out_t[:, s, :], tmp, x_t[:, s, :])

            nc.sync.dma_start(out_v[blk], out_t)


def _split_excess_waits(nc: bass.Bass, cap: int = 1) -> None:
    """Walrus's per-instruction sync encoders take few wait slots (the TT
    struct rejects 2+). Move extra semaphore waits onto preceding NoOps on
    the same engine; engine program order preserves the semantics."""
    counter = [0]
    for f in nc.m.functions:
        for blk in f.blocks:
            il = blk.instructions
            out = []
            changed = False
            for ins in il:
                si = ins.sync_info
                if si is not None and len(si.on_wait) > cap:
                    extra = list(si.on_wait[:-cap]) if cap else list(si.on_wait)
                    keep = list(si.on_wait[-cap:]) if cap else []
                    for w in extra:
                        nop = mybir.InstNoOp(name=f"NOPW-{counter[0]}")
                        counter[0] += 1
                        nop.engine = ins.engine
                        nop.sync_info = mybir.SyncInfo(on_wait=[w], on_update=[])
                        nc.register_instruction(nop)
                        out.append(nop)
                    ins.sync_info = mybir.SyncInfo(on_wait=keep,
                                                   on_update=list(si.on_update))
                    changed = True
                out.append(ins)
            if changed:
                blk.instructions = out


def build_module() -> bass.Bass:
    nc = bass.Bass("TRN2", target_bir_lowering=False, debug=False)
    x_d = nc.dram_tensor("x", [BL, D], F32, kind="ExternalInput").ap()
    x0_d = nc.dram_tensor("x0", [BL, D], BF16, kind="ExternalInput").ap()
    w1_d = nc.dram_tensor("w1", [KC, P, ES], BF16, kind="ExternalInput").ap()
    wg_d = nc.dram_tensor("wg", [KC, P, E], BF16, kind="ExternalInput").ap()
    ut_d = nc.dram_tensor("ut", [NPAIR, P, D], BF16, kind="ExternalInput").ap()
    b_d = nc.dram_tensor("bexp", [E, D], BF16, kind="ExternalInput").ap()
    bg_d = nc.dram_tensor("bg", [1, E], BF16, kind="ExternalInput").ap()
    out_d = nc.dram_tensor("out", [BL, D], F32, kind="ExternalOutput").ap()
    with tile.TileContext(nc) as tc:
        _kernel_body(tc, out_d, x_d, x0_d, w1_d, wg_d, ut_d, b_d, bg_d)
    _split_excess_waits(nc)
    return nc


_NC_CACHE: bass.Bass | None = None


def _get_module() -> bass.Bass:
    global _NC_CACHE
    if _NC_CACHE is None:
        _NC_CACHE = build_module()
    return _NC_CACHE


def make_in_maps(x0, x, U, V, C, b, Wg, bg):
    x0 = np.asarray(x0, dtype=np.float32)
    x = np.asarray(x, dtype=np.float32)
    U = np.asarray(U, dtype=np.float32)
    V = np.asarray(V, dtype=np.float32)
    C = np.asarray(C, dtype=np.float32)
    b = np.asarray(b, dtype=np.float32)
    Wg = np.asarray(Wg, dtype=np.float32)
    bg = np.asarray(bg, dtype=np.float32)

    # W1[e] = V[e].T @ C[e]  -> [D, E*R] column-chunked by contraction chunk
    w1 = np.einsum("erd,ers->eds", V, C).transpose(1, 0, 2).reshape(D, ES)
    w1 = np.ascontiguousarray(w1.reshape(KC, P, ES)).astype(NPBF16)
    wg = np.ascontiguousarray(Wg.T.reshape(KC, P, E)).astype(NPBF16)
    # UT pairs: rows 0-63 = U[2p].T, rows 64-127 = U[2p+1].T
    ut = np.ascontiguousarray(U.transpose(0, 2, 1).reshape(NPAIR, P, D)).astype(NPBF16)
    b_bf = b.astype(NPBF16)
    bg_bf = bg.reshape(1, E).astype(NPBF16)
    x0_bf = x0.astype(NPBF16)

    in_maps = []
    for c in range(NCORES):
        sl = slice(c * BL, (c + 1) * BL)
        in_maps.append({
            "x": np.ascontiguousarray(x[sl]),
            "x0": np.ascontiguousarray(x0_bf[sl]),
            "w1": w1, "wg": wg, "ut": ut, "bexp": b_bf, "bg": bg_bf,
        })
    return in_maps


def kernel(x0, x, U, V, C, b, Wg, bg, _trace=False, _trace_kwargs=None):
    nc = _get_module()
    in_maps = make_in_maps(x0, x, U, V, C, b, Wg, bg)
    res = run_bass_kernel_spmd(nc, in_maps, list(range(NCORES)),
                               trace=_trace, **(_trace_kwargs or {}))
    out = np.concatenate([res.results[c]["out"] for c in range(NCORES)], axis=0)
    if _trace:
        return out, res
    return out


if __name__ == "__main__":
    rng = np.random.default_rng(0)
    ins = {
        "x0": rng.standard_normal((B, D), dtype=np.float32),
        "x": rng.standard_normal((B, D), dtype=np.float32),
        "U": (rng.standard_normal((E, D, R)) * 0.02).astype(np.float32),
        "V": (rng.standard_normal((E, R, D)) * 0.02).astype(np.float32),
        "C": (rng.standard_normal((E, R, R)) * 0.02).astype(np.float32),
        "b": np.zeros((E, D), np.float32),
        "Wg": (rng.standard_normal((E, D)) * 0.02).astype(np.float32),
        "bg": np.zeros((E,), np.float32),
    }
    out = kernel(**ins)
    print("out", out.shape, out.dtype)


# revision 12
# speedup vs baseline: 1.8936x; 1.8936x over previous
"""Trainium2 Bass kernel for nn_CrossLayerV2 (MoE low-rank bilinear cross layer).

Computes, for x0,x [B,D], U [E,D,R], V [E,R,D], C [E,R,R], b [E,D], Wg [E,D], bg [E]:
    g    = softmax(x @ Wg.T + bg, axis=1)                 # [B, E]
    xvc  = einsum('bd,eds->bes', x, W1)  with W1[e] = V[e].T @ C[e]   (host-fused)
    out  = x0 * einsum('be,bes,eds->bd', g, xvc, U) + g @ b + x * g.sum(1, keepdims=True)
Note g.sum(1) == 1 (softmax), so the last term is exactly x.

Strategy: data-parallel over batch across 8 NeuronCores (params replicated).
Per core: 2048 rows = 4 blocks x (4 subtiles of 128 rows). All matmuls in bf16
with fp32 PSUM accumulation; the final residual add is fp32. x arrives twice:
fp32 (for the exact +x term) and bf16 (DMA-transposed into [D, B] layout for
the tensor-engine contractions).
"""

import sys

for _p in ("/opt/trn_rl_repo", "/opt/pypackages"):
    if _p not in sys.path:
        sys.path.append(_p)

from contextlib import ExitStack

import ml_dtypes
import numpy as np

import concourse.bass as bass
import concourse.tile as tile
from concourse import mybir
from concourse.bass_utils import run_bass_kernel_spmd
from concourse.masks import make_identity

BF16 = mybir.dt.bfloat16
F32 = mybir.dt.float32
NPBF16 = ml_dtypes.bfloat16

B, D, R, E = 16384, 512, 64, 8
NCORES = 8
BL = B // NCORES          # rows per core (2048)
P = 128                   # partitions
NSUB = 4                  # subtiles per block
BLOCK = NSUB * P          # rows per block (512)
NBLK = BL // BLOCK        # blocks per core (4)
KC = D // P               # contraction chunks (4)
NPAIR = E // 2            # expert pairs (4)
ES = E * R                # expert-packed width (512)


def _expand_gate_ap(g_ap: bass.AP, reps: int) -> bass.AP:
    """[P, E] gate AP -> broadcast [P, E, reps] AP (stride-0 inner dim)."""
    return bass.AP(tensor=g_ap.tensor, offset=g_ap.offset,
                   ap=[g_ap.ap[0], g_ap.ap[1], [0, reps]])


def _kernel_body(tc, out_d, x_d, x0_d, xtd_d, w1_d, wg_d, ut_d, b_d, bg_d):
    nc = tc.nc
    x_v = x_d.rearrange("(nb s p) d -> nb p s d", s=NSUB, p=P)
    x0_v = x0_d.rearrange("(nb s p) d -> nb p s d", s=NSUB, p=P)
    out_v = out_d.rearrange("(nb s p) d -> nb p s d", s=NSUB, p=P)

    with ExitStack() as ctx:
        const = ctx.enter_context(tc.tile_pool(name="const", bufs=1))
        io = ctx.enter_context(tc.tile_pool(name="io", bufs=2))
        outp = ctx.enter_context(tc.tile_pool(name="outp", bufs=2))
        work = ctx.enter_context(tc.tile_pool(name="work", bufs=2))
        small = ctx.enter_context(tc.tile_pool(name="small", bufs=4))
        ps_gate = ctx.enter_context(tc.tile_pool(name="ps_gate", bufs=2, space="PSUM"))
        ps_xvc = ctx.enter_context(tc.tile_pool(name="ps_xvc", bufs=2, space="PSUM"))
        ps_tp = ctx.enter_context(tc.tile_pool(name="ps_tp", bufs=2, space="PSUM"))
        ps_out = ctx.enter_context(tc.tile_pool(name="ps_out", bufs=2, space="PSUM"))

        # --- resident constants / params ---
        ident = const.tile([P, P], BF16)
        make_identity(nc, ident)
        ones_row = const.tile([1, BLOCK], BF16)
        nc.vector.memset(ones_row, 1.0)
        w1_sb = const.tile([P, KC, ES], BF16)
        nc.sync.dma_start(w1_sb, w1_d.rearrange("k p n -> p k n"))
        wg_sb = const.tile([P, KC, E], BF16)
        nc.sync.dma_start(wg_sb, wg_d.rearrange("k p n -> p k n"))
        ut_sb = const.tile([P, NPAIR, D], BF16)
        nc.sync.dma_start(ut_sb, ut_d.rearrange("k p n -> p k n"))
        b_sb = const.tile([E, D], BF16)
        nc.sync.dma_start(b_sb, b_d)
        bg_sb = const.tile([1, E], BF16)
        nc.sync.dma_start(bg_sb, bg_d)

        for blk in range(NBLK):
            x_t = io.tile([P, NSUB, D], F32, tag="x")
            nc.sync.dma_start(x_t, x_v[blk])
            x0_t = io.tile([P, NSUB, D], BF16, tag="x0")
            nc.sync.dma_start(x0_t, x0_v[blk])
            # x.T shard (host-pretransposed, bf16): [dd, k, block rows]
            xt = work.tile([P, KC, BLOCK], BF16, tag="xt")
            nc.sync.dma_start(
                xt, xtd_d.rearrange("(k p) b -> p k b", p=P)[:, :,
                    blk * BLOCK:(blk + 1) * BLOCK])

            out_t = outp.tile([P, NSUB, D], F32, tag="o")

            def head(s):
                """xvc matmuls + gate logits for subtile s."""
                ssl = slice(s * P, (s + 1) * P)
                p_xvc = ps_xvc.tile([P, ES], F32, tag="xvc")
                for k in range(KC):
                    nc.tensor.matmul(p_xvc, xt[:, k, ssl], w1_sb[:, k, :],
                                     start=(k == 0), stop=(k == KC - 1))
                # gate logits, transposed: lgt[e, j] = sum_d Wg[e,d] x[j,d] + bg
                p_lgt = ps_gate.tile([E, P], F32, tag="gate")
                for k in range(KC):
                    nc.tensor.matmul(p_lgt, wg_sb[:, k, :], xt[:, k, ssl],
                                     start=(k == 0), stop=False)
                nc.tensor.matmul(p_lgt, bg_sb, ones_row[:, :P],
                                 start=False, stop=True)
                lgt_sb = small.tile([E, P], BF16, tag="lgt")
                nc.vector.tensor_copy(lgt_sb, p_lgt)
                p_log = ps_gate.tile([P, E], BF16, tag="gate")
                nc.tensor.transpose(p_log, lgt_sb, ident[:E, :E])
                return p_xvc, p_log

            state = {0: head(0)}

            for s in range(NSUB):
                ssl = slice(s * P, (s + 1) * P)
                p_xvc, p_log = state.pop(s)
                # keep PE fed: queue the next subtile's matmuls ahead of this
                # subtile's tail (which stalls PE behind DVE/ACT handoffs)
                if s + 1 < NSUB:
                    state[s + 1] = head(s + 1)

                # softmax over E (free dim), produced directly in bf16
                nm = small.tile([P, 1], F32, tag="nm")
                nc.vector.reduce_max(nm, p_log, axis=mybir.AxisListType.X,
                                     negate=True)
                p_t = small.tile([P, E], F32, tag="pt")
                s_t = small.tile([P, 1], F32, tag="st")
                nc.scalar.activation(p_t, p_log, mybir.ActivationFunctionType.Exp,
                                     bias=nm[:, :], accum_out=s_t[:, :])
                r_t = small.tile([P, 1], F32, tag="rt")
                nc.vector.reciprocal(r_t, s_t)
                g_bf = small.tile([P, E], BF16, tag="g")
                nc.vector.tensor_scalar_mul(g_bf, p_t, r_t[:, :])

                # gxvc = xvc * g (gate broadcast along the rank dim), cast bf16
                gxvc = work.tile([P, ES], BF16, tag="gxvc")
                nc.vector.tensor_tensor(gxvc, p_xvc, _expand_gate_ap(g_bf[:, :], R),
                                        op=mybir.AluOpType.mult)

                # transpose expert-pair blocks + the gate row for g @ b
                p_gx = ps_tp.tile([P, ES], BF16, tag="tp")
                for pr in range(NPAIR):
                    nc.tensor.transpose(p_gx[:, pr * P:(pr + 1) * P],
                                        gxvc[:, pr * P:(pr + 1) * P], ident)
                p_gt = ps_tp.tile([E, P], BF16, tag="tp")
                nc.tensor.transpose(p_gt, g_bf, ident)
                gxt = work.tile([P, ES], BF16, tag="gxt")
                nc.scalar.copy(gxt, p_gx)
                gt_sb = small.tile([E, P], BF16, tag="gt")
                nc.scalar.copy(gt_sb, p_gt)

                # gproj = sum over expert pairs + g @ b
                p_o = ps_out.tile([P, D], F32, tag="out")
                for pr in range(NPAIR):
                    nc.tensor.matmul(p_o, gxt[:, pr * P:(pr + 1) * P],
                                     ut_sb[:, pr, :], start=(pr == 0), stop=False)
                nc.tensor.matmul(p_o, gt_sb, b_sb, start=False, stop=True)

                # out = x0 * gproj + x  (fp32 residual)
                tmp = work.tile([P, D], F32, tag="tmp")
                nc.vector.tensor_mul(tmp, p_o, x0_t[:, s, :])
                nc.gpsimd.tensor_add(out_t[:, s, :], tmp, x_t[:, s, :])

            nc.sync.dma_start(out_v[blk], out_t)


def _split_excess_waits(nc: bass.Bass, cap: int = 1) -> None:
    """Walrus's per-instruction sync encoders take few wait slots (the TT
    struct rejects 2+). Move extra semaphore waits onto preceding NoOps on
    the same engine; engine program order preserves the semantics."""
    counter = [0]
    for f in nc.m.functions:
        for blk in f.blocks:
            il = blk.instructions
            out = []
            changed = False
            for ins in il:
                si = ins.sync_info
                if si is not None and len(si.on_wait) > cap:
                    extra = list(si.on_wait[:-cap]) if cap else list(si.on_wait)
                    keep = list(si.on_wait[-cap:]) if cap else []
                    for w in extra:
                        nop = mybir.InstNoOp(name=f"NOPW-{counter[0]}")
                        counter[0] += 1
                        nop.engine = ins.engine
                        nop.sync_info = mybir.SyncInfo(on_wait=[w], on_update=[])
                        nc.register_instruction(nop)
                        out.append(nop)
                    ins.sync_info = mybir.SyncInfo(on_wait=keep,
                                                   on_update=list(si.on_update))
                    changed = True
                out.append(ins)
            if changed:
                blk.instructions = out


def build_module() -> bass.Bass:
    nc = bass.Bass("TRN2", target_bir_lowering=False, debug=False)
    x_d = nc.dram_tensor("x", [BL, D], F32, kind="ExternalInput").ap()
    x0_d = nc.dram_tensor("x0", [BL, D], BF16, kind="ExternalInput").ap()
    xtd_d = nc.dram_tensor("xtd", [D, BL], BF16, kind="ExternalInput").ap()
    w1_d = nc.dram_tensor("w1", [KC, P, ES], BF16, kind="ExternalInput").ap()
    wg_d = nc.dram_tensor("wg", [KC, P, E], BF16, kind="ExternalInput").ap()
    ut_d = nc.dram_tensor("ut", [NPAIR, P, D], BF16, kind="ExternalInput").ap()
    b_d = nc.dram_tensor("bexp", [E, D], BF16, kind="ExternalInput").ap()
    bg_d = nc.dram_tensor("bg", [1, E], BF16, kind="ExternalInput").ap()
    out_d = nc.dram_tensor("out", [BL, D], F32, kind="ExternalOutput").ap()
    with tile.TileContext(nc) as tc:
        _kernel_body(tc, out_d, x_d, x0_d, xtd_d, w1_d, wg_d, ut_d, b_d, bg_d)
    _split_excess_waits(nc)
    return nc


_NC_CACHE: bass.Bass | None = None


def _get_module() -> bass.Bass:
    global _NC_CACHE
    if _NC_CACHE is None:
        _NC_CACHE = build_module()
    return _NC_CACHE


def make_in_maps(x0, x, U, V, C, b, Wg, bg):
    x0 = np.asarray(x0, dtype=np.float32)
    x = np.asarray(x, dtype=np.float32)
    U = np.asarray(U, dtype=np.float32)
    V = np.asarray(V, dtype=np.float32)
    C = np.asarray(C, dtype=np.float32)
    b = np.asarray(b, dtype=np.float32)
    Wg = np.asarray(Wg, dtype=np.float32)
    bg = np.asarray(bg, dtype=np.float32)

    # W1[e] = V[e].T @ C[e]  -> [D, E*R] column-chunked by contraction chunk
    w1 = np.einsum("erd,ers->eds", V, C).transpose(1, 0, 2).reshape(D, ES)
    w1 = np.ascontiguousarray(w1.reshape(KC, P, ES)).astype(NPBF16)
    wg = np.ascontiguousarray(Wg.T.reshape(KC, P, E)).astype(NPBF16)
    # UT pairs: rows 0-63 = U[2p].T, rows 64-127 = U[2p+1].T
    ut = np.ascontiguousarray(U.transpose(0, 2, 1).reshape(NPAIR, P, D)).astype(NPBF16)
    b_bf = b.astype(NPBF16)
    bg_bf = bg.reshape(1, E).astype(NPBF16)
    x0_bf = x0.astype(NPBF16)
    xt_all = np.ascontiguousarray(x.T.astype(NPBF16))  # [D, B]

    in_maps = []
    for c in range(NCORES):
        sl = slice(c * BL, (c + 1) * BL)
        in_maps.append({
            "x": np.ascontiguousarray(x[sl]),
            "x0": np.ascontiguousarray(x0_bf[sl]),
            "xtd": np.ascontiguousarray(xt_all[:, sl]),
            "w1": w1, "wg": wg, "ut": ut, "bexp": b_bf, "bg": bg_bf,
        })
    return in_maps


def kernel(x0, x, U, V, C, b, Wg, bg, _trace=False, _trace_kwargs=None):
    nc = _get_module()
    in_maps = make_in_maps(x0, x, U, V, C, b, Wg, bg)
    res = run_bass_kernel_spmd(nc, in_maps, list(range(NCORES)),
                               trace=_trace, **(_trace_kwargs or {}))
    out = np.concatenate([res.results[c]["out"] for c in range(NCORES)], axis=0)
    if _trace:
        return out, res
    return out


if __name__ == "__main__":
    rng = np.random.default_rng(0)
    ins = {
        "x0": rng.standard_normal((B, D), dtype=np.float32),
        "x": rng.standard_normal((B, D), dtype=np.float32),
        "U": (rng.standard_normal((E, D, R)) * 0.02).astype(np.float32),
        "V": (rng.standard_normal((E, R, D)) * 0.02).astype(np.float32),
        "C": (rng.standard_normal((E, R, R)) * 0.02).astype(np.float32),
        "b": np.zeros((E, D), np.float32),
        "Wg": (rng.standard_normal((E, D)) * 0.02).astype(np.float32),
        "bg": np.zeros((E,), np.float32),
    }
    out = kernel(**ins)
    print("out", out.shape, out.dtype)
